# revision 6
# baseline (speedup 1.0000x reference)
"""Trainium2 Bass kernel for the Performer-style random-feature map:

    out[n, s] = exp(-||x_n||^2 / 2) * S^{-1/2} * exp((x @ W.T)[n, s] + b[s])
              = exp((x @ W.T)[n, s] - 0.5*||x_n||^2 - 0.5*ln(S)) * exp(b[s])

Sharding: data-parallel over the N (row) axis across 8 NeuronCores; W and b
replicated.  Each core computes a [2048, 2048] output block.  Pure SPMD, no
collectives.

v3 (fp8 DoubleRow, 3 DMA rings):
  - matmul in fp8e4 with perf_mode=DoubleRow: 256-deep contraction per
    instruction at the same 216ns issue gap as bf16 -> half the PE time.
    W is pre-scaled by 32 on the host so its values sit in e4m3's normal
    range; the 1/32 is folded into the ACT exp scale.
  - per [128, 2048] row block: 16 DoubleRow matmuls fill 4 PSUM banks
    (k2-outer so the stationary x-block strip is reused 4x), then two
    half-block ACT exp(psum/32 + bias_n) -> bf16 and DVE multiplies by
    exp(b), DMA out.  PSUM ping-pongs 2 x 4 banks.
  - each hardware DMA queue sustains only ~200 GB/s, so traffic is spread
    over three rings: sync = x fp8 strips + even-block outputs, scalar =
    W fp8 strips + b broadcast + odd-block outputs, gpsimd = xn row
    stream for the norm path.
  - row-norm bias via DVE square/reduce/affine (tensor_tensor_reduce
    would fuse this but dies on HW with an INTERNAL error).
  - output is bf16 on device (<=0.4% quantization, far under the 2e-2
    gate), widened to f32 on the host during the gather.
"""

import sys
from contextlib import ExitStack

if "/opt/trn_rl_repo" not in sys.path:
    sys.path.insert(0, "/opt/trn_rl_repo")

import numpy as np

import concourse.bacc as bacc
import concourse.bass as bass
import concourse.tile as tile
from concourse import mybir

P = 128          # SBUF partitions
N_FULL = 16384   # total rows
D_FULL = 1024    # contraction dim
S_FULL = 2048    # output features
N_CORES = 8
NC_FULL = N_FULL // N_CORES  # rows per core
W_SCALE = 32.0   # host pre-scale on W so fp8 e4m3 sees ~N(0,1) values

F32 = mybir.dt.float32
BF16 = mybir.dt.bfloat16
F8 = mybir.dt.float8e4
DR = mybir.MatmulPerfMode.DoubleRow


def build_nc(NCc=NC_FULL, D=D_FULL, S=S_FULL, warmup=24, xn_ahead=4,
             bias_ahead=2, half_act=True):
    """Build the single-core Bass program (same program runs SPMD on 8 cores)."""
    nc = bacc.Bacc("TRN2", target_bir_lowering=False, debug=False)

    xT = nc.dram_tensor("xT8", [D, NCc], F8, kind="ExternalInput").ap()
    xn = nc.dram_tensor("xn", [NCc, D], BF16, kind="ExternalInput").ap()
    w = nc.dram_tensor("w8", [D, S], F8, kind="ExternalInput").ap()
    bv = nc.dram_tensor("bias", [S], BF16, kind="ExternalInput").ap()
    out = nc.dram_tensor("out", [NCc, S], BF16, kind="ExternalOutput").ap()

    KT = D // P            # 8 k strips of 128
    K2 = KT // 2           # 4 DoubleRow chunks of 256
    NB = NCc // P          # 128-row output blocks
    NS = 512               # matmul moving free dim (one PSUM bank fp32)
    SH = S // NS           # s-chunks per row block
    S2 = S // 2 if half_act else S   # ACT/mult/out granularity
    H2 = S // S2
    neg_half_ln_s = float(-0.5 * np.log(S))

    with tile.TileContext(nc) as tc, ExitStack() as ctx:
        singles = ctx.enter_context(tc.tile_pool(name="singles", bufs=1))
        w_sb = singles.tile([P, KT, S], F8)
        x_sb = singles.tile([P, KT, NCc], F8)
        b_bc = singles.tile([P, S], BF16)
        eb = singles.tile([P, S], BF16)
        bias_tiles = [
            singles.tile([P, 1], F32, tag=f"bias{nb}", name=f"bias{nb}")
            for nb in range(NB)
        ]

        # warm-up dummies (no DMA dependency -> PE starts immediately)
        dx = singles.tile([P, 2, P], F8)
        dw = singles.tile([P, 2, NS], F8)
        nc.vector.memset(dx, 0.0)
        nc.vector.memset(dw, 0.0)

        xn_pool = ctx.enter_context(tc.tile_pool(name="xnp", bufs=5))
        sq_pool = ctx.enter_context(tc.tile_pool(name="sqp", bufs=2))
        r_pool = ctx.enter_context(tc.tile_pool(name="rp", bufs=4))
        psum_pool = ctx.enter_context(
            tc.tile_pool(name="psum", bufs=2, space="PSUM"))
        tmp_pool = ctx.enter_context(tc.tile_pool(name="tmp", bufs=4))
        out_pool = ctx.enter_context(tc.tile_pool(name="osb", bufs=4))

        # scalar ring: W k-pair chunks, then the b broadcast
        wr = w.rearrange("(k p) s -> p k s", p=P)
        for k2 in range(K2):
            nc.scalar.dma_start(w_sb[:, 2 * k2:2 * k2 + 2, :],
                                wr[:, 2 * k2:2 * k2 + 2, :])
        bv_bcast = bass.AP(tensor=bv.tensor, offset=bv.offset,
                           ap=[[0, P]] + list(bv.ap))
        nc.scalar.dma_start(b_bc, bv_bcast)
        nc.scalar.activation(eb, b_bc, func=mybir.ActivationFunctionType.Exp)

        # sync ring: x k-pair chunks (outputs join later)
        xr = xT.rearrange("(k p) n -> p k n", p=P)
        for k2 in range(K2):
            nc.sync.dma_start(x_sb[:, 2 * k2:2 * k2 + 2, :],
                              xr[:, 2 * k2:2 * k2 + 2, :])

        # gpsimd ring: xn row stream for the norm path
        xn_tiles = {}

        def load_xn(nb):
            xt = xn_pool.tile([P, D], BF16, tag="xns", name=f"xn{nb}")
            nc.gpsimd.dma_start(xt, xn[nb * P:(nb + 1) * P, :])
            xn_tiles[nb] = xt

        def r_bias(nb):
            # bias_n = -0.5*||x_n||^2 - 0.5*ln(S)
            xt = xn_tiles.pop(nb)
            sq = sq_pool.tile([P, D], BF16)
            nc.vector.tensor_mul(sq, xt, xt)
            r_raw = r_pool.tile([P, 1], F32)
            nc.vector.tensor_reduce(
                r_raw, sq, axis=mybir.AxisListType.X, op=mybir.AluOpType.add)
            nc.vector.tensor_scalar(
                out=bias_tiles[nb], in0=r_raw,
                scalar1=-0.5, scalar2=neg_half_ln_s,
                op0=mybir.AluOpType.mult, op1=mybir.AluOpType.add)

        for nb in range(min(xn_ahead, NB)):
            load_xn(nb)
        for nb in range(min(bias_ahead, NB)):
            r_bias(nb)

        # keep the PE busy (and HAM-warm) while the operand strips stream in
        for i in range(warmup):
            wps = psum_pool.tile([P, S], F32, tag="ps", name=f"warm{i}")
            nc.tensor.matmul(wps[:, 0:NS], lhsT=dx, rhs=dw,
                             start=True, stop=True, perf_mode=DR)

        for nb in range(NB):
            if nb + xn_ahead < NB:
                load_xn(nb + xn_ahead)
            ps = psum_pool.tile([P, S], F32, tag="ps", name=f"ps{nb}")
            for k2 in range(K2):
                lt = x_sb[:, 2 * k2:2 * k2 + 2, nb * P:(nb + 1) * P]
                for h in range(SH):
                    nc.tensor.matmul(
                        ps[:, h * NS:(h + 1) * NS],
                        lhsT=lt,
                        rhs=w_sb[:, 2 * k2:2 * k2 + 2, h * NS:(h + 1) * NS],
                        start=(k2 == 0),
                        stop=(k2 == K2 - 1),
                        perf_mode=DR)
            o_sb = out_pool.tile([P, S], BF16)
            for h2 in range(H2):
                sl = slice(h2 * S2, (h2 + 1) * S2)
                tmp = tmp_pool.tile([P, S2], BF16)
                nc.scalar.activation(
                    tmp, ps[:, sl],
                    func=mybir.ActivationFunctionType.Exp,
                    bias=bias_tiles[nb],
                    scale=1.0 / W_SCALE)
                nc.vector.tensor_mul(o_sb[:, sl], tmp, eb[:, sl])
            # alternate output rings: each queue sustains only ~200 GB/s
            eng = nc.sync if nb % 2 == 0 else nc.scalar
            eng.dma_start(out[nb * P:(nb + 1) * P, :], o_sb)
            if nb + bias_ahead < NB:
                r_bias(nb + bias_ahead)

    nc.compile()
    return nc


_NC_CACHE = {}


def _get_nc(**kwargs):
    key = tuple(sorted(kwargs.items()))
    if key not in _NC_CACHE:
        _NC_CACHE[key] = build_nc(**kwargs)
    return _NC_CACHE[key]


def make_in_maps(x, W, b):
    import ml_dtypes
    bf16 = ml_dtypes.bfloat16
    f8 = ml_dtypes.float8_e4m3
    w8 = np.ascontiguousarray(
        (W.T.astype(np.float32) * W_SCALE).astype(f8))
    bf = np.ascontiguousarray(b.astype(bf16))
    in_maps = []
    for i in range(N_CORES):
        xs = np.ascontiguousarray(
            x[i * NC_FULL:(i + 1) * NC_FULL].astype(np.float32))
        in_maps.append({
            "xT8": np.ascontiguousarray(xs.T.astype(f8)),
            "xn": np.ascontiguousarray(xs.astype(bf16)),
            "w8": w8,
            "bias": bf,
        })
    return in_maps


def run_hw(x, W, b, trace=False, **build_kwargs):
    """Run on 8 NeuronCores; returns (out [N, S] f32, BassKernelResults)."""
    from concourse.bass_utils import run_bass_kernel_spmd
    from concourse.bass_interp import get_hw_module

    nc = _get_nc(**build_kwargs)
    in_maps = make_in_maps(x, W, b)
    old_m = nc.m
    nc.m = get_hw_module(nc.m)
    try:
        res = run_bass_kernel_spmd(
            nc, in_maps, core_ids=list(range(N_CORES)), trace=trace)
    finally:
        nc.m = old_m
    out = np.concatenate(
        [res.results[i]["out"].astype(np.float32) for i in range(N_CORES)],
        axis=0)
    return out, res


def kernel(x, W, b):
    out, _ = run_hw(x, W, b, trace=False)
    return out
